# revision 2
# baseline (speedup 1.0000x reference)
"""Trainium2 Bass kernel for multi-head attention (nn_AttentionWithDropout) v4.

Reference computation (fp32):
    q = query @ Wq.T + bq ; k = key @ Wk.T + bk ; v = value @ Wv.T + bv
    per head: P = softmax(q k^T / sqrt(E)) ; o = P v
    out = concat_heads(o) @ Wo.T + bo

Sharding (8 cores): data-parallel over batch (2 groups of 4 cores) x
tensor-parallel over heads (4 heads / 256 channels per core).

v3 over v2: normalization moved to AFTER the AllGather.  The block drain
is now just DVE copies (unnormalized aoT + raw row-sums, both bf16) so
the PE never waits on the slow [1,512] DVE reciprocal at block
boundaries (v2 lost ~10us/block to that, chilling the HAM clock gate).
The out-projection side computes one [16,512] reciprocal per token chunk
(off the critical path, >=2 blocks after the AllGather fired) and
re-scales the gathered aoT tiles via a tiny block-diagonal broadcast
matmul + DVE multiply.  Startup DMAs reordered: wk + first xk chunk
first, consts after.
"""

import os
import sys

sys.path.insert(0, "/opt/trn_rl_repo")

import numpy as np

# ---- problem constants (hardcoded per the harness contract) ----
B, L, E = 2, 2048, 1024
H, D = 16, 64
N_CORES = 8
TP = 4                  # cores per batch group (head-parallel)
CH = E // TP            # 256 channels (4 heads) per core
SCALE = 1.0 / 32.0      # 1/sqrt(E)
KT = E // 128           # 8 contraction tiles for projections
NKT = L // 128          # 16 key-token tiles
NTC = L // 512          # 4 query/token chunks
AGR = CH + 4            # AllGather rows: 256 aoT + 4 sums

FILL_PER_ROUND = int(os.environ.get("KERNEL_FILLS", "2"))


def _split_multi_waits(nc):
    """The nix walrus in this container only encodes one semaphore wait per
    instruction.  Hoist extras into standalone InstEventSemaphore waits."""
    from concourse import mybir

    n_split = 0
    for fn in nc.m.functions:
        for bb in fn.blocks:
            out = []
            for inst in bb.instructions:
                si = inst.sync_info
                if si is not None and si.on_wait and len(si.on_wait) > 1:
                    waits = list(si.on_wait)
                    for k, w in enumerate(waits[:-1]):
                        wi = mybir.InstEventSemaphore(
                            name=f"{inst.name}-hw{k}", ins=[], outs=[])
                        wi.engine = inst.engine
                        wi.debug = inst.debug
                        wi.sync_info = mybir.SyncInfo(on_wait=[w],
                                                      on_update=[])
                        out.append(wi)
                        n_split += 1
                    si.on_wait = [waits[-1]]
                out.append(inst)
            bb.instructions[:] = out
    return n_split


def _build_nc():
    import contextlib

    import concourse.bass as bass
    import concourse.tile as tile
    from concourse import mybir

    f32 = mybir.dt.float32
    f32r = mybir.dt.float32r
    bf16 = mybir.dt.bfloat16
    AF = mybir.ActivationFunctionType

    nc = bass.Bass("TRN2", target_bir_lowering=False, debug=False,
                   num_devices=N_CORES)
    lp = contextlib.ExitStack()
    lp.enter_context(nc.allow_low_precision(
        reason="bf16/f32r storage is deliberate; error budget measured "
               "2.5e-3 vs 2e-2 gate"))

    # ---- per-core external IO ----
    xqT = nc.dram_tensor("xqT", [E, L], bf16, kind="ExternalInput")
    xkT_d = nc.dram_tensor("xkT", [E, L], bf16, kind="ExternalInput")
    xvT_d = nc.dram_tensor("xvT", [E, L], bf16, kind="ExternalInput")
    # weights host-pretiled: [128, KT*CH]; block kt at cols [kt*CH,(kt+1)*CH)
    # holds W.T rows [kt*128,(kt+1)*128) -- 4KB contiguous per partition line
    wqT = nc.dram_tensor("wqT", [128, KT * CH], bf16, kind="ExternalInput")
    wkT = nc.dram_tensor("wkT", [128, KT * CH], bf16, kind="ExternalInput")
    wvT = nc.dram_tensor("wvT", [128, KT * CH], bf16, kind="ExternalInput")
    bqc = nc.dram_tensor("bqc", [CH], f32, kind="ExternalInput")
    bkc = nc.dram_tensor("bkc", [CH], f32, kind="ExternalInput")
    bvb = nc.dram_tensor("bvb", [128, CH], f32, kind="ExternalInput")
    woTb = nc.dram_tensor("woTb", [128, KT * CH], bf16, kind="ExternalInput")
    bor = nc.dram_tensor("bor", [128, CH], f32, kind="ExternalInput")
    onescol = nc.dram_tensor("onescol", [128, 4], bf16, kind="ExternalInput")
    bd16 = nc.dram_tensor("bd16", [16, 128 * KT], bf16,
                          kind="ExternalInput")
    out = nc.dram_tensor("out", [L, CH], f32, kind="ExternalOutput")

    groups = [[0, 1, 2, 3], [4, 5, 6, 7]]

    with tile.TileContext(nc) as tc:
        with (
            tc.tile_pool(name="consts", bufs=1) as consts,
            tc.tile_pool(name="persist", bufs=1) as persist,
            tc.tile_pool(name="dram", bufs=1, space="DRAM") as dpool,
        ):
            agi = [dpool.tile([AGR, 512], bf16, name=f"agi{q}")
                   for q in range(NTC)]
            ago = [dpool.tile([TP, AGR, 512], bf16, name=f"ago{q}")
                   for q in range(NTC)]

            # ---- persistent SBUF tensors ----
            qT = [persist.tile([128, L], bf16, name=f"qT{i}") for i in range(2)]
            kTt = [persist.tile([128, L], bf16, name=f"kT{i}")
                   for i in range(2)]
            v_sb = [persist.tile([128, 4, 65], bf16, name=f"v{t}")
                    for t in range(NKT)]
            wq_w = persist.tile([128, KT * CH], bf16, name="wqw")
            woT_w = persist.tile([128, KT * CH], bf16, name="wow")
            wq_sb = [wq_w[:, i * CH:(i + 1) * CH] for i in range(KT)]
            woT_sb = [woT_w[:, i * CH:(i + 1) * CH] for i in range(KT)]
            # all of xq lives on-chip: 8 wide tiles, one 4KB-line DMA each
            xq_w = [persist.tile([128, L], bf16, name=f"xqw{i}")
                    for i in range(KT)]
            xq = {c: [xq_w[kt][:, c * 512:(c + 1) * 512] for kt in range(KT)]
                  for c in range(NTC)}
            ones4 = consts.tile([128, 4], bf16)

            bo_bcast = consts.tile([128, CH], f32)
            bvb_sb = consts.tile([128, CH], f32)
            bd16_sb = consts.tile([16, 128 * KT], bf16)
            bias_cols = {}
            for nm in ("q", "k"):
                for ct in range(2):
                    bias_cols[(nm, ct)] = consts.tile([128, 1], f32,
                                                      name=f"b{nm}{ct}")

            # ================= phase A: k/v/q0 projections =================
            with (
                tc.tile_pool(name="xpool", bufs=32) as xpool,
                tc.tile_pool(name="wkv", bufs=2) as wkv,
                tc.tile_pool(name="ppj", bufs=2, space="PSUM") as ppj,
                tc.tile_pool(name="ppv", bufs=2, space="PSUM") as ppv,
            ):
                # phase-A-critical DMAs first: wk (one 4KB-line dma), then
                # the first wide xk chunk; small consts after
                wk_w = wkv.tile([128, KT * CH], bf16, name="wc")
                nc.sync.dma_start(wk_w[:], wkT[:])
                wk = [wk_w[:, i * CH:(i + 1) * CH] for i in range(KT)]
                xk_t = {}
                xv_t = {}

                def load_x(dst, src, c2):
                    tiles = []
                    for kt in range(KT):
                        t = xpool.tile([128, 1024], bf16, name="xc")
                        nc.sync.dma_start(
                            t[:], src[kt * 128:(kt + 1) * 128,
                                      c2 * 1024:(c2 + 1) * 1024])
                        tiles.append(t)
                    dst[c2] = tiles

                load_x(xk_t, xkT_d, 0)
                wv_w = wkv.tile([128, KT * CH], bf16, name="wc")
                nc.sync.dma_start(wv_w[:], wvT[:])
                wv = [wv_w[:, i * CH:(i + 1) * CH] for i in range(KT)]
                for ct in range(2):
                    nc.sync.dma_start(
                        bias_cols[("k", ct)][:],
                        bkc[ct * 128:(ct + 1) * 128].unsqueeze(1))
                    nc.sync.dma_start(
                        bias_cols[("q", ct)][:],
                        bqc[ct * 128:(ct + 1) * 128].unsqueeze(1))
                nc.sync.dma_start(bvb_sb[:], bvb[:])
                nc.sync.dma_start(bo_bcast[:], bor[:])
                nc.sync.dma_start(bd16_sb[:], bd16[:])
                nc.sync.dma_start(ones4[:], onescol[:])
                load_x(xv_t, xvT_d, 0)
                load_x(xk_t, xkT_d, 1)
                load_x(xv_t, xvT_d, 1)
                nc.sync.dma_start(wq_w[:], wqT[:])
                for kt in range(KT):
                    nc.sync.dma_start(xq_w[kt][:],
                                      xqT[kt * 128:(kt + 1) * 128, :])

                for c in range(NTC):
                    cs = slice(c * 512, (c + 1) * 512)
                    c2, ch = divmod(c, 2)
                    xk = [t[:, ch * 512:(ch + 1) * 512] for t in xk_t[c2]]
                    xv = [t[:, ch * 512:(ch + 1) * 512] for t in xv_t[c2]]
                    # k projection -> kTt[ct][:, tokens c]
                    for ct in range(2):
                        ps = ppj.tile([128, 512], f32, name="pj")
                        for kt in range(KT):
                            nc.tensor.matmul(
                                ps[:], wk[kt][:, ct * 128:(ct + 1) * 128],
                                xk[kt], start=(kt == 0),
                                stop=(kt == KT - 1))
                        nc.vector.tensor_scalar_add(
                            kTt[ct][:, cs], ps[:], bias_cols[("k", ct)][:])
                    # v projection, direct [token, chan] layout
                    for tt in range(4):
                        ti = c * 4 + tt
                        ps = ppv.tile([128, CH], f32, name="pv")
                        for kt in range(KT):
                            nc.tensor.matmul(
                                ps[:],
                                xv[kt][:, tt * 128:(tt + 1) * 128],
                                wv[kt], start=(kt == 0),
                                stop=(kt == KT - 1))
                        nc.vector.tensor_add(
                            v_sb[ti][:, :, 0:64],
                            ps.rearrange("p (h d) -> p h d", h=4),
                            bvb_sb.rearrange("p (h d) -> p h d", h=4))
                        nc.vector.tensor_copy(v_sb[ti][:, :, 64:65],
                                              ones4.unsqueeze(2))
                # q projection chunk 0
                for ct in range(2):
                    ps = ppj.tile([128, 512], f32, name="pj")
                    for kt in range(KT):
                        nc.tensor.matmul(
                            ps[:], wq_sb[kt][:, ct * 128:(ct + 1) * 128],
                            xq[0][kt], start=(kt == 0), stop=(kt == KT - 1))
                    nc.vector.tensor_scalar_add(
                        qT[ct][:, 0:512], ps[:], bias_cols[("q", ct)][:])

            # ================= phase B: attention + fills =================
            with (
                tc.tile_pool(name="upool", bufs=3) as upool,
                tc.tile_pool(name="agp", bufs=16) as agp,
                tc.tile_pool(name="s16p", bufs=2) as s16p,
                tc.tile_pool(name="invp", bufs=2) as invp,
                tc.tile_pool(name="aop", bufs=4) as aop,
                tc.tile_pool(name="smp", bufs=4) as smp,
                tc.tile_pool(name="opool", bufs=3) as opool,
                tc.tile_pool(name="stp", bufs=2, space="PSUM") as stp,
                tc.tile_pool(name="pacc", bufs=3, space="PSUM") as pacc,
                tc.tile_pool(name="pfill", bufs=1, space="PSUM") as pfill,
            ):
                nc.sync.dma_start(woT_w[:], woTb[:])

                # ---- fill-task machinery: lists of one-shot closures ----
                fill_steps = []

                def push_qproj(c):
                    for ct in range(2):
                        state = {}

                        def mk(k, c=c, ct=ct, state=state):
                            def go():
                                if k == 0:
                                    state["ps"] = pfill.tile(
                                        [128, 512], f32, name="fps")
                                nc.tensor.matmul(
                                    state["ps"][:],
                                    wq_sb[k][:, ct * 128:(ct + 1) * 128],
                                    xq[c][k], start=(k == 0),
                                    stop=(k == KT - 1))
                            return go

                        def evac(c=c, ct=ct, state=state):
                            nc.vector.tensor_scalar_add(
                                qT[ct][:, c * 512:(c + 1) * 512],
                                state["ps"][:], bias_cols[("q", ct)][:])
                        for k in range(KT):
                            fill_steps.append(mk(k))
                        fill_steps.append(evac)

                agt = {}
                s16s = {}
                inv16 = {}

                def push_agload(qc):
                    """DMA-only step: pull the gathered tiles + sums into
                    SBUF.  Scheduled a block after the AllGather fired so
                    nothing engine-side ever waits on the collective."""
                    def prep_load(qc=qc):
                        tiles = []
                        s16 = s16p.tile([16, 512], bf16, name="s16")
                        for src in range(TP):
                            for h2 in range(2):
                                t = agp.tile([128, 512], bf16, name="ag")
                                nc.sync.dma_start(
                                    t[:], ago[qc][src,
                                                  h2 * 128:(h2 + 1) * 128, :])
                                tiles.append(t)
                            nc.sync.dma_start(
                                s16[src * 4:(src + 1) * 4, :],
                                ago[qc][src, CH:AGR, :])
                        agt[qc] = tiles
                        s16s[qc] = s16
                    fill_steps.append(prep_load)

                def push_outproj(qc, tail=False):
                    """Reciprocal + rescale + 4 output chains."""
                    def prep_recip(qc=qc):
                        iv = invp.tile([16, 512], bf16, name="iv")
                        nc.vector.reciprocal(iv[:], s16s[qc][:])
                        inv16[qc] = iv
                    fill_steps.append(prep_recip)

                    def mk_scale(k, qc=qc):
                        def go():
                            if tail and k % 2 == 0:
                                sp = stp.tile([128, 1024], f32, name="st")
                                sp = sp[0:128, 0:512]
                            else:
                                sp = pfill.tile([128, 512], f32, name="fps")
                                sp = sp[:]
                            nc.tensor.matmul(
                                sp, bd16_sb[:, k * 128:(k + 1) * 128],
                                inv16[qc][:], start=True, stop=True)
                            nc.vector.tensor_mul(agt[qc][k][:],
                                                 agt[qc][k][:], sp)
                        return go

                    for k in range(KT):
                        fill_steps.append(mk_scale(k))

                    for ti in range(4):
                        state = {}

                        def mk(k, qc=qc, ti=ti, state=state):
                            def go():
                                if k == 0:
                                    if tail and ti % 2 == 0:
                                        t = stp.tile([128, 1024], f32,
                                                     name="st")
                                        state["ps"] = t[0:128, 0:512]
                                    else:
                                        t = pfill.tile([128, 512], f32,
                                                       name="fps")
                                        state["ps"] = t[:]
                                nc.tensor.matmul(
                                    state["ps"][:, 0:CH],
                                    agt[qc][k][:, ti * 128:(ti + 1) * 128],
                                    woT_sb[k][:], start=(k == 0),
                                    stop=(k == KT - 1))
                            return go

                        def evac(qc=qc, ti=ti, state=state):
                            ob = opool.tile([128, CH], f32, name="ob")
                            nc.vector.tensor_add(ob[:], state["ps"][:, 0:CH],
                                                 bo_bcast[:])
                            row0 = qc * 512 + ti * 128
                            nc.sync.dma_start(out[row0:row0 + 128, :], ob[:])
                        for k in range(KT):
                            fill_steps.append(mk(k))
                        fill_steps.append(evac)

                def drain_block(qc, hp, accs):
                    """Copy out unnormalized aoT + raw sums (both bf16)."""
                    for j in range(2):
                        hl = 2 * hp + j
                        aoT = aop.tile([64, 512], bf16, name="aot")
                        nc.vector.tensor_copy(aoT[:], accs[j][0:64, :])
                        sm = smp.tile([1, 512], bf16, name="sm")
                        nc.vector.tensor_copy(sm[:], accs[j][64:65, :])
                        nc.sync.dma_start(
                            agi[qc][hl * 64:(hl + 1) * 64, :], aoT[:])
                        nc.sync.dma_start(agi[qc][CH + hl:CH + hl + 1, :],
                                          sm[:])

                def emit_ag(qc):
                    nc.gpsimd.collective_compute(
                        "AllGather", mybir.AluOpType.bypass,
                        replica_groups=groups,
                        ins=[agi[qc].opt()], outs=[ago[qc].opt()])

                # fill-task schedule per block index bi = qc*2+hp.
                # out-proj for qc is pushed only when its AllGather (fired
                # at block 2qc+2 round 0) has surely landed, so no PE fill
                # step ever waits on an in-flight collective.
                on_block_start = {
                    0: lambda: push_qproj(1),
                    1: lambda: push_qproj(2),
                    2: lambda: push_qproj(3),
                    3: lambda: push_agload(0),
                    4: lambda: push_outproj(0),
                    5: lambda: push_agload(1),
                    6: lambda: push_outproj(1),
                    7: lambda: push_agload(2),
                }

                prev_drain = None      # (qc, hp, accs) of the previous block
                for qc in range(NTC):
                    qs = slice(qc * 512, (qc + 1) * 512)
                    for hp in range(2):
                        bi = qc * 2 + hp
                        if bi in on_block_start:
                            on_block_start[bi]()
                        accs = [pacc.tile([65, 512], f32, name="acc")
                                for _ in range(2)]
                        u_prev = None
                        for r in range(NKT + 1):
                            if r < NKT:
                                st = stp.tile([128, 1024], f32, name="st")
                                for j in range(2):
                                    nc.tensor.matmul(
                                        st[:, j * 512:(j + 1) * 512],
                                        kTt[hp][j * 64:(j + 1) * 64,
                                                r * 128:(r + 1) * 128],
                                        qT[hp][j * 64:(j + 1) * 64, qs],
                                        start=True, stop=True)
                                u = upool.tile([128, 1024], bf16, name="u")
                                nc.scalar.activation(u[:], st[:], AF.Exp,
                                                     scale=SCALE)
                            if r == 0 and prev_drain is not None:
                                drain_block(*prev_drain)
                                if prev_drain[1] == 1:
                                    emit_ag(prev_drain[0])
                                prev_drain = None
                            if r >= 1:
                                rp = r - 1
                                for j in range(2):
                                    nc.tensor.matmul(
                                        accs[j][:],
                                        v_sb[rp][:, 2 * hp + j, :],
                                        u_prev[:, j * 512:(j + 1) * 512],
                                        start=(rp == 0), stop=(rp == NKT - 1))
                            if 1 <= r <= NKT - 1:
                                for _ in range(FILL_PER_ROUND):
                                    if fill_steps:
                                        fill_steps.pop(0)()
                            u_prev = u
                        prev_drain = (qc, hp, accs)

                # tail: drain + final AllGather first, then out-proj(2)
                # runs while AG_3 is in flight, then out-proj(3)
                drain_block(*prev_drain)
                emit_ag(3)
                push_outproj(2, tail=True)
                push_agload(3)
                push_outproj(3, tail=True)
                while fill_steps:
                    fill_steps.pop(0)()

    _split_multi_waits(nc)
    return nc


_NC_CACHE = {}


def _get_nc():
    key = (FILL_PER_ROUND,)
    if key not in _NC_CACHE:
        _NC_CACHE[key] = _build_nc()
    return _NC_CACHE[key]


def kernel(query, key, value, Wq, bq, Wk, bk, Wv, bv, Wo, bo,
           _trace=False, _trace_cores=None):
    from concourse.bass_utils import run_bass_kernel_spmd
    import ml_dtypes

    bf = ml_dtypes.bfloat16
    query = np.asarray(query, dtype=np.float32)
    key = np.asarray(key, dtype=np.float32)
    value = np.asarray(value, dtype=np.float32)
    Wq = np.asarray(Wq, dtype=np.float32)
    bq = np.asarray(bq, dtype=np.float32)
    Wk = np.asarray(Wk, dtype=np.float32)
    bk = np.asarray(bk, dtype=np.float32)
    Wv = np.asarray(Wv, dtype=np.float32)
    bv = np.asarray(bv, dtype=np.float32)
    Wo = np.asarray(Wo, dtype=np.float32)
    bo = np.asarray(bo, dtype=np.float32)

    nc = _get_nc()

    xT = {b: {"q": np.ascontiguousarray(query[b].T.astype(bf)),
              "k": np.ascontiguousarray(key[b].T.astype(bf)),
              "v": np.ascontiguousarray(value[b].T.astype(bf))}
          for b in range(B)}

    def tile_w(w):  # [CH, E] slice of W -> [128, KT*CH] pretiled .T blocks
        wt = np.ascontiguousarray(w.T.astype(bf))          # [E, CH]
        return np.ascontiguousarray(
            wt.reshape(KT, 128, CH).transpose(1, 0, 2).reshape(128, KT * CH))

    bd16_np = np.zeros((16, 128 * KT), dtype=bf)
    for k in range(KT):
        src, h2 = divmod(k, 2)
        for c in range(128):
            bd16_np[src * 4 + h2 * 2 + c // 64, k * 128 + c] = 1

    in_maps = []
    for c in range(N_CORES):
        b, g = divmod(c, TP)
        sl = slice(g * CH, (g + 1) * CH)
        in_maps.append({
            "xqT": xT[b]["q"], "xkT": xT[b]["k"], "xvT": xT[b]["v"],
            "wqT": tile_w(Wq[sl, :]), "wkT": tile_w(Wk[sl, :]),
            "wvT": tile_w(Wv[sl, :]),
            "bqc": bq[sl], "bkc": bk[sl],
            "bvb": np.ascontiguousarray(
                np.broadcast_to(bv[sl].reshape(1, CH), (128, CH))),
            "woTb": tile_w(Wo[sl, :]),
            "bor": np.ascontiguousarray(
                np.broadcast_to(bo[sl].reshape(1, CH), (128, CH))),
            "onescol": np.ones((128, 4), dtype=bf),
            "bd16": bd16_np,
        })

    kwargs = {}
    if _trace:
        kwargs.update(trace=True,
                      trace_cores=_trace_cores or list(range(N_CORES)))
    res = run_bass_kernel_spmd(nc, in_maps, core_ids=list(range(N_CORES)),
                               **kwargs)

    full = np.empty((B, L, E), dtype=np.float32)
    for c in range(N_CORES):
        b, g = divmod(c, TP)
        full[b, :, g * CH:(g + 1) * CH] = res.results[c]["out"]

    if _trace:
        kernel.last_exec_ns = res.exec_time_ns
        kernel.last_results = res
    return full


# revision 3
# speedup vs baseline: 1.1879x; 1.1879x over previous
"""Trainium2 Bass kernel for multi-head attention (nn_AttentionWithDropout) v8.

Reference computation (fp32):
    q = query @ Wq.T + bq ; k = key @ Wk.T + bk ; v = value @ Wv.T + bv
    per head: P = softmax(q k^T / sqrt(E)) ; o = P v
    out = concat_heads(o) @ Wo.T + bo

Sharding (8 cores): data-parallel over batch (2 groups of 4 cores) x
tensor-parallel over heads (4 heads / 256 channels per core).

v3 over v2: normalization moved to AFTER the AllGather.  The block drain
is now just DVE copies (unnormalized aoT + raw row-sums, both bf16) so
the PE never waits on the slow [1,512] DVE reciprocal at block
boundaries (v2 lost ~10us/block to that, chilling the HAM clock gate).
The out-projection side computes one [16,512] reciprocal per token chunk
(off the critical path, >=2 blocks after the AllGather fired) and
re-scales the gathered aoT tiles via a tiny block-diagonal broadcast
matmul + DVE multiply.  Startup DMAs reordered: wk + first xk chunk
first, consts after.
"""

import os
import sys

sys.path.insert(0, "/opt/trn_rl_repo")

import numpy as np

# ---- problem constants (hardcoded per the harness contract) ----
B, L, E = 2, 2048, 1024
H, D = 16, 64
N_CORES = 8
TP = 4                  # cores per batch group (head-parallel)
CH = E // TP            # 256 channels (4 heads) per core
SCALE = 1.0 / 32.0      # 1/sqrt(E)
KT = E // 128           # 8 contraction tiles for projections
NKT = L // 128          # 16 key-token tiles
NTC = L // 512          # 4 query/token chunks
AGR = CH + 4            # AllGather rows: 256 aoT + 4 sums

FILL_PER_ROUND = int(os.environ.get("KERNEL_FILLS", "2"))


def _split_multi_waits(nc):
    """The nix walrus in this container only encodes one semaphore wait per
    instruction.  Hoist extras into standalone InstEventSemaphore waits."""
    from concourse import mybir

    n_split = 0
    for fn in nc.m.functions:
        for bb in fn.blocks:
            out = []
            for inst in bb.instructions:
                si = inst.sync_info
                if si is not None and si.on_wait and len(si.on_wait) > 1:
                    waits = list(si.on_wait)
                    for k, w in enumerate(waits[:-1]):
                        wi = mybir.InstEventSemaphore(
                            name=f"{inst.name}-hw{k}", ins=[], outs=[])
                        wi.engine = inst.engine
                        wi.debug = inst.debug
                        wi.sync_info = mybir.SyncInfo(on_wait=[w],
                                                      on_update=[])
                        out.append(wi)
                        n_split += 1
                    si.on_wait = [waits[-1]]
                out.append(inst)
            bb.instructions[:] = out
    return n_split


def _build_nc():
    import contextlib

    import concourse.bass as bass
    import concourse.tile as tile
    from concourse import mybir

    f32 = mybir.dt.float32
    f32r = mybir.dt.float32r
    bf16 = mybir.dt.bfloat16
    AF = mybir.ActivationFunctionType

    nc = bass.Bass("TRN2", target_bir_lowering=False, debug=False,
                   num_devices=N_CORES)
    lp = contextlib.ExitStack()
    lp.enter_context(nc.allow_low_precision(
        reason="bf16/f32r storage is deliberate; error budget measured "
               "2.5e-3 vs 2e-2 gate"))

    # ---- per-core external IO ----
    xqT = nc.dram_tensor("xqT", [E, L], bf16, kind="ExternalInput")
    xkT_d = nc.dram_tensor("xkT", [E, L], bf16, kind="ExternalInput")
    xvT_d = nc.dram_tensor("xvT", [E, L], bf16, kind="ExternalInput")
    # weights host-pretiled: [128, KT*CH]; block kt at cols [kt*CH,(kt+1)*CH)
    # holds W.T rows [kt*128,(kt+1)*128) -- 4KB contiguous per partition line
    wqT = nc.dram_tensor("wqT", [128, KT * CH], bf16, kind="ExternalInput")
    wkT = nc.dram_tensor("wkT", [128, KT * CH], bf16, kind="ExternalInput")
    wvT = nc.dram_tensor("wvT", [128, KT * CH], bf16, kind="ExternalInput")
    bqc = nc.dram_tensor("bqc", [CH], f32, kind="ExternalInput")
    bkc = nc.dram_tensor("bkc", [CH], f32, kind="ExternalInput")
    bvb = nc.dram_tensor("bvb", [128, CH], f32, kind="ExternalInput")
    woTb = nc.dram_tensor("woTb", [128, KT * CH], bf16, kind="ExternalInput")
    bor = nc.dram_tensor("bor", [128, CH], f32, kind="ExternalInput")
    onescol = nc.dram_tensor("onescol", [128, 4], bf16, kind="ExternalInput")
    bd16 = nc.dram_tensor("bd16", [16, 128 * KT], bf16,
                          kind="ExternalInput")
    out = nc.dram_tensor("out", [L, CH], f32, kind="ExternalOutput")

    groups = [[0, 1, 2, 3], [4, 5, 6, 7]]

    with tile.TileContext(nc) as tc:
        with (
            tc.tile_pool(name="consts", bufs=1) as consts,
            tc.tile_pool(name="persist", bufs=1) as persist,
            tc.tile_pool(name="dram", bufs=1, space="DRAM") as dpool,
        ):
            agi = [dpool.tile([AGR, 512], bf16, name=f"agi{q}")
                   for q in range(3)]
            ago = [dpool.tile([TP, AGR, 512], bf16, name=f"ago{q}")
                   for q in range(3)]
            # last chunk: sums go in two tiny early AllGathers so the
            # reciprocal is done before the aoT gather lands
            agi3s = [dpool.tile([2, 512], bf16, name=f"agi3s{h}")
                     for h in range(2)]
            ago3s = [dpool.tile([TP, 2, 512], bf16, name=f"ago3s{h}")
                     for h in range(2)]
            agi3ao = dpool.tile([CH, 512], bf16, name="agi3ao")
            ago3ao = dpool.tile([TP, CH, 512], bf16, name="ago3ao")

            # ---- persistent SBUF tensors ----
            qT = [persist.tile([128, L], bf16, name=f"qT{i}") for i in range(2)]
            kTt = [persist.tile([128, L], bf16, name=f"kT{i}")
                   for i in range(2)]
            v_sb = [persist.tile([128, 4, 65], bf16, name=f"v{t}")
                    for t in range(NKT)]
            wq_w = persist.tile([128, KT * CH], bf16, name="wqw")
            woT_w = persist.tile([128, KT * CH], bf16, name="wow")
            wq_sb = [wq_w[:, i * CH:(i + 1) * CH] for i in range(KT)]
            woT_sb = [woT_w[:, i * CH:(i + 1) * CH] for i in range(KT)]
            # all of xq lives on-chip: 8 wide tiles, one 4KB-line DMA each
            xq_w = [persist.tile([128, L], bf16, name=f"xqw{i}")
                    for i in range(KT)]
            xq = {c: [xq_w[kt][:, c * 512:(c + 1) * 512] for kt in range(KT)]
                  for c in range(NTC)}
            ones4 = consts.tile([128, 4], bf16)

            bo_bcast = consts.tile([128, CH], f32)
            bvb_sb = consts.tile([128, CH], f32)
            bd16_sb = consts.tile([16, 128 * KT], bf16)
            bias_cols = {}
            for nm in ("q", "k"):
                for ct in range(2):
                    bias_cols[(nm, ct)] = consts.tile([128, 1], f32,
                                                      name=f"b{nm}{ct}")

            # ================= phase A: k/v/q0 projections =================
            with (
                tc.tile_pool(name="xpool", bufs=32) as xpool,
                tc.tile_pool(name="wkv", bufs=2) as wkv,
                tc.tile_pool(name="ppj", bufs=2, space="PSUM") as ppj,
                tc.tile_pool(name="ppv", bufs=2, space="PSUM") as ppv,
            ):
                # phase-A-critical DMAs first: wk (one 4KB-line dma), then
                # the first wide xk chunk; small consts after
                wk_w = wkv.tile([128, KT * CH], bf16, name="wc")
                nc.sync.dma_start(wk_w[:], wkT[:])
                wk = [wk_w[:, i * CH:(i + 1) * CH] for i in range(KT)]
                xk_t = {}
                xv_t = {}

                def load_x(dst, src, c2):
                    tiles = []
                    for kt in range(KT):
                        t = xpool.tile([128, 1024], bf16, name="xc")
                        nc.sync.dma_start(
                            t[:], src[kt * 128:(kt + 1) * 128,
                                      c2 * 1024:(c2 + 1) * 1024])
                        tiles.append(t)
                    dst[c2] = tiles

                load_x(xk_t, xkT_d, 0)
                wv_w = wkv.tile([128, KT * CH], bf16, name="wc")
                nc.sync.dma_start(wv_w[:], wvT[:])
                wv = [wv_w[:, i * CH:(i + 1) * CH] for i in range(KT)]
                for ct in range(2):
                    nc.sync.dma_start(
                        bias_cols[("k", ct)][:],
                        bkc[ct * 128:(ct + 1) * 128].unsqueeze(1))
                    nc.sync.dma_start(
                        bias_cols[("q", ct)][:],
                        bqc[ct * 128:(ct + 1) * 128].unsqueeze(1))
                nc.sync.dma_start(bvb_sb[:], bvb[:])
                nc.sync.dma_start(bo_bcast[:], bor[:])
                nc.sync.dma_start(bd16_sb[:], bd16[:])
                nc.sync.dma_start(ones4[:], onescol[:])
                load_x(xv_t, xvT_d, 0)
                load_x(xk_t, xkT_d, 1)
                load_x(xv_t, xvT_d, 1)
                nc.sync.dma_start(wq_w[:], wqT[:])
                for kt in range(KT):
                    nc.sync.dma_start(xq_w[kt][:],
                                      xqT[kt * 128:(kt + 1) * 128, :])

                for c in range(NTC):
                    cs = slice(c * 512, (c + 1) * 512)
                    c2, ch = divmod(c, 2)
                    xk = [t[:, ch * 512:(ch + 1) * 512] for t in xk_t[c2]]
                    xv = [t[:, ch * 512:(ch + 1) * 512] for t in xv_t[c2]]
                    # k projection -> kTt[ct][:, tokens c]
                    for ct in range(2):
                        ps = ppj.tile([128, 512], f32, name="pj")
                        for kt in range(KT):
                            nc.tensor.matmul(
                                ps[:], wk[kt][:, ct * 128:(ct + 1) * 128],
                                xk[kt], start=(kt == 0),
                                stop=(kt == KT - 1))
                        nc.vector.tensor_scalar_add(
                            kTt[ct][:, cs], ps[:], bias_cols[("k", ct)][:])
                    # v projection, direct [token, chan] layout
                    for tt in range(4):
                        ti = c * 4 + tt
                        ps = ppv.tile([128, CH], f32, name="pv")
                        for kt in range(KT):
                            nc.tensor.matmul(
                                ps[:],
                                xv[kt][:, tt * 128:(tt + 1) * 128],
                                wv[kt], start=(kt == 0),
                                stop=(kt == KT - 1))
                        nc.vector.tensor_add(
                            v_sb[ti][:, :, 0:64],
                            ps.rearrange("p (h d) -> p h d", h=4),
                            bvb_sb.rearrange("p (h d) -> p h d", h=4))
                        nc.vector.tensor_copy(v_sb[ti][:, :, 64:65],
                                              ones4.unsqueeze(2))
                # q projection chunk 0
                for ct in range(2):
                    ps = ppj.tile([128, 512], f32, name="pj")
                    for kt in range(KT):
                        nc.tensor.matmul(
                            ps[:], wq_sb[kt][:, ct * 128:(ct + 1) * 128],
                            xq[0][kt], start=(kt == 0), stop=(kt == KT - 1))
                    nc.vector.tensor_scalar_add(
                        qT[ct][:, 0:512], ps[:], bias_cols[("q", ct)][:])

            # ================= phase B: attention + fills =================
            with (
                tc.tile_pool(name="upool", bufs=3) as upool,
                tc.tile_pool(name="agp", bufs=16) as agp,
                tc.tile_pool(name="s16p", bufs=2) as s16p,
                tc.tile_pool(name="invp", bufs=2) as invp,
                tc.tile_pool(name="aop", bufs=4) as aop,
                tc.tile_pool(name="smp", bufs=4) as smp,
                tc.tile_pool(name="opool", bufs=3) as opool,
                tc.tile_pool(name="stp", bufs=2, space="PSUM") as stp,
                tc.tile_pool(name="pacc", bufs=3, space="PSUM") as pacc,
                tc.tile_pool(name="pfill", bufs=1, space="PSUM") as pfill,
            ):
                nc.sync.dma_start(woT_w[:], woTb[:])

                # ---- fill-task machinery: lists of one-shot closures ----
                fill_steps = []

                def push_qproj(c):
                    for ct in range(2):
                        state = {}

                        def mk(k, c=c, ct=ct, state=state):
                            def go():
                                if k == 0:
                                    state["ps"] = pfill.tile(
                                        [128, 512], f32, name="fps")
                                nc.tensor.matmul(
                                    state["ps"][:],
                                    wq_sb[k][:, ct * 128:(ct + 1) * 128],
                                    xq[c][k], start=(k == 0),
                                    stop=(k == KT - 1))
                            return go

                        def evac(c=c, ct=ct, state=state):
                            nc.vector.tensor_scalar_add(
                                qT[ct][:, c * 512:(c + 1) * 512],
                                state["ps"][:], bias_cols[("q", ct)][:])
                        for k in range(KT):
                            fill_steps.append(mk(k))
                        fill_steps.append(evac)

                agt = {}
                s16s = {}
                inv16 = {}

                def push_agload(qc):
                    """DMA-only step: pull the gathered tiles + sums into
                    SBUF.  Scheduled a block after the AllGather fired so
                    nothing engine-side ever waits on the collective."""
                    def prep_load(qc=qc):
                        tiles = []
                        s16 = s16p.tile([16, 512], bf16, name="s16")
                        for src in range(TP):
                            for h2 in range(2):
                                t = agp.tile([128, 512], bf16, name="ag")
                                nc.sync.dma_start(
                                    t[:], ago[qc][src,
                                                  h2 * 128:(h2 + 1) * 128, :])
                                tiles.append(t)
                            nc.sync.dma_start(
                                s16[src * 4:(src + 1) * 4, :],
                                ago[qc][src, CH:AGR, :])
                        agt[qc] = tiles
                        s16s[qc] = s16
                    fill_steps.append(prep_load)

                def push_outproj(qc, tail=False):
                    """Reciprocal + rescale + 4 output chains."""
                    def prep_recip(qc=qc):
                        iv = invp.tile([16, 512], bf16, name="iv")
                        nc.vector.reciprocal(iv[:], s16s[qc][:])
                        inv16[qc] = iv
                    fill_steps.append(prep_recip)

                    def mk_scale(k, qc=qc):
                        def go():
                            if tail and k % 2 == 0:
                                sp = stp.tile([128, 1024], f32, name="st")
                                sp = sp[0:128, 0:512]
                            else:
                                sp = pfill.tile([128, 512], f32, name="fps")
                                sp = sp[:]
                            nc.tensor.matmul(
                                sp, bd16_sb[:, k * 128:(k + 1) * 128],
                                inv16[qc][:], start=True, stop=True)
                            nc.vector.tensor_mul(agt[qc][k][:],
                                                 agt[qc][k][:], sp)
                        return go

                    for k in range(KT):
                        fill_steps.append(mk_scale(k))

                    for ti in range(4):
                        state = {}

                        def mk(k, qc=qc, ti=ti, state=state):
                            def go():
                                if k == 0:
                                    if tail and ti % 2 == 0:
                                        t = stp.tile([128, 1024], f32,
                                                     name="st")
                                        state["ps"] = t[0:128, 0:512]
                                    else:
                                        t = pfill.tile([128, 512], f32,
                                                       name="fps")
                                        state["ps"] = t[:]
                                nc.tensor.matmul(
                                    state["ps"][:, 0:CH],
                                    agt[qc][k][:, ti * 128:(ti + 1) * 128],
                                    woT_sb[k][:], start=(k == 0),
                                    stop=(k == KT - 1))
                            return go

                        def evac(qc=qc, ti=ti, state=state):
                            ob = opool.tile([128, CH], f32, name="ob")
                            nc.vector.tensor_add(ob[:], state["ps"][:, 0:CH],
                                                 bo_bcast[:])
                            row0 = qc * 512 + ti * 128
                            nc.sync.dma_start(out[row0:row0 + 128, :], ob[:])
                        for k in range(KT):
                            fill_steps.append(mk(k))
                        fill_steps.append(evac)

                def drain_block(qc, hp, accs):
                    """Copy out unnormalized aoT + raw sums (both bf16)."""
                    for j in range(2):
                        hl = 2 * hp + j
                        aoT = aop.tile([64, 512], bf16, name="aot")
                        nc.vector.tensor_copy(aoT[:], accs[j][0:64, :])
                        sm = smp.tile([1, 512], bf16, name="sm")
                        nc.vector.tensor_copy(sm[:], accs[j][64:65, :])
                        if qc == 3:
                            nc.sync.dma_start(
                                agi3ao[hl * 64:(hl + 1) * 64, :], aoT[:])
                            nc.sync.dma_start(agi3s[hp][j:j + 1, :], sm[:])
                        else:
                            nc.sync.dma_start(
                                agi[qc][hl * 64:(hl + 1) * 64, :], aoT[:])
                            nc.sync.dma_start(
                                agi[qc][CH + hl:CH + hl + 1, :], sm[:])

                def emit_ag(qc, hp=1):
                    if qc == 3:
                        # sums half fires immediately (8KB, lands fast)
                        nc.gpsimd.collective_compute(
                            "AllGather", mybir.AluOpType.bypass,
                            replica_groups=groups,
                            ins=[agi3s[hp].opt()], outs=[ago3s[hp].opt()])
                        if hp == 1:
                            nc.gpsimd.collective_compute(
                                "AllGather", mybir.AluOpType.bypass,
                                replica_groups=groups,
                                ins=[agi3ao.opt()], outs=[ago3ao.opt()])
                    elif hp == 1:
                        nc.gpsimd.collective_compute(
                            "AllGather", mybir.AluOpType.bypass,
                            replica_groups=groups,
                            ins=[agi[qc].opt()], outs=[ago[qc].opt()])

                # fill-task schedule per block index bi = qc*2+hp.
                # out-proj for qc is pushed only when its AllGather (fired
                # at block 2qc+2 round 0) has surely landed, so no PE fill
                # step ever waits on an in-flight collective.
                on_block_start = {
                    0: lambda: push_qproj(1),
                    1: lambda: push_qproj(2),
                    2: lambda: push_qproj(3),
                    3: lambda: push_agload(0),
                    4: lambda: push_outproj(0),
                    5: lambda: push_agload(1),
                    6: lambda: push_outproj(1),
                    7: lambda: push_agload(2),
                }

                prev_drain = None      # (qc, hp, accs) of the previous block
                for qc in range(NTC):
                    qs = slice(qc * 512, (qc + 1) * 512)
                    for hp in range(2):
                        bi = qc * 2 + hp
                        if bi in on_block_start:
                            on_block_start[bi]()
                        accs = [pacc.tile([65, 512], f32, name="acc")
                                for _ in range(2)]
                        u_prev = None
                        for r in range(NKT + 1):
                            if r < NKT:
                                st = stp.tile([128, 1024], f32, name="st")
                                for j in range(2):
                                    nc.tensor.matmul(
                                        st[:, j * 512:(j + 1) * 512],
                                        kTt[hp][j * 64:(j + 1) * 64,
                                                r * 128:(r + 1) * 128],
                                        qT[hp][j * 64:(j + 1) * 64, qs],
                                        start=True, stop=True)
                                u = upool.tile([128, 1024], bf16, name="u")
                                nc.scalar.activation(u[:], st[:], AF.Exp,
                                                     scale=SCALE)
                            if r == 0 and prev_drain is not None:
                                drain_block(*prev_drain)
                                emit_ag(prev_drain[0], prev_drain[1])
                                prev_drain = None
                            if r >= 1:
                                rp = r - 1
                                for j in range(2):
                                    nc.tensor.matmul(
                                        accs[j][:],
                                        v_sb[rp][:, 2 * hp + j, :],
                                        u_prev[:, j * 512:(j + 1) * 512],
                                        start=(rp == 0), stop=(rp == NKT - 1))
                            if 1 <= r <= NKT - 1:
                                for _ in range(FILL_PER_ROUND):
                                    if fill_steps:
                                        fill_steps.pop(0)()
                            u_prev = u
                        prev_drain = (qc, hp, accs)

                # ---- tail ----
                # drain fires the tiny sums gather + the aoT gather;
                # out-proj(2) occupies the PE while they fly; the qc=3
                # reciprocal runs off the sums gather during that, so after
                # the aoT gather only loads + rescale + chains remain.
                drain_block(*prev_drain)
                emit_ag(3, 1)
                push_outproj(2, tail=True)
                while fill_steps:
                    fill_steps.pop(0)()

                s16_3 = s16p.tile([16, 512], bf16, name="s16")
                for src in range(TP):
                    nc.sync.dma_start(s16_3[src * 4:src * 4 + 2, :],
                                      ago3s[0][src, :, :])
                    nc.sync.dma_start(s16_3[src * 4 + 2:src * 4 + 4, :],
                                      ago3s[1][src, :, :])
                iv3 = invp.tile([16, 512], bf16, name="iv")
                nc.vector.reciprocal(iv3[:], s16_3[:])

                agt3 = []
                for src in range(TP):
                    for h2 in range(2):
                        t = agp.tile([128, 512], bf16, name="ag")
                        nc.sync.dma_start(
                            t[:], ago3ao[src, h2 * 128:(h2 + 1) * 128, :])
                        agt3.append(t)
                for k in range(KT):
                    if k % 2 == 0:
                        sp = stp.tile([128, 1024], f32, name="st")
                        sp = sp[0:128, 0:512]
                    else:
                        sp = pfill.tile([128, 512], f32, name="fps")
                        sp = sp[:]
                    nc.tensor.matmul(sp, bd16_sb[:, k * 128:(k + 1) * 128],
                                     iv3[:], start=True, stop=True)
                    nc.vector.tensor_mul(agt3[k][:], agt3[k][:], sp)
                for ti in range(4):
                    if ti % 2 == 0:
                        t = stp.tile([128, 1024], f32, name="st")
                        ps3 = t[0:128, 0:512]
                    else:
                        t = pfill.tile([128, 512], f32, name="fps")
                        ps3 = t[:]
                    for k in range(KT):
                        nc.tensor.matmul(
                            ps3[:, 0:CH],
                            agt3[k][:, ti * 128:(ti + 1) * 128],
                            woT_sb[k][:], start=(k == 0), stop=(k == KT - 1))
                    ob = opool.tile([128, CH], f32, name="ob")
                    nc.vector.tensor_add(ob[:], ps3[:, 0:CH], bo_bcast[:])
                    row0 = 3 * 512 + ti * 128
                    nc.sync.dma_start(out[row0:row0 + 128, :], ob[:])

    _split_multi_waits(nc)
    return nc


_NC_CACHE = {}


def _get_nc():
    key = (FILL_PER_ROUND,)
    if key not in _NC_CACHE:
        _NC_CACHE[key] = _build_nc()
    return _NC_CACHE[key]


def kernel(query, key, value, Wq, bq, Wk, bk, Wv, bv, Wo, bo,
           _trace=False, _trace_cores=None):
    from concourse.bass_utils import run_bass_kernel_spmd
    import ml_dtypes

    bf = ml_dtypes.bfloat16
    query = np.asarray(query, dtype=np.float32)
    key = np.asarray(key, dtype=np.float32)
    value = np.asarray(value, dtype=np.float32)
    Wq = np.asarray(Wq, dtype=np.float32)
    bq = np.asarray(bq, dtype=np.float32)
    Wk = np.asarray(Wk, dtype=np.float32)
    bk = np.asarray(bk, dtype=np.float32)
    Wv = np.asarray(Wv, dtype=np.float32)
    bv = np.asarray(bv, dtype=np.float32)
    Wo = np.asarray(Wo, dtype=np.float32)
    bo = np.asarray(bo, dtype=np.float32)

    nc = _get_nc()

    xT = {b: {"q": np.ascontiguousarray(query[b].T.astype(bf)),
              "k": np.ascontiguousarray(key[b].T.astype(bf)),
              "v": np.ascontiguousarray(value[b].T.astype(bf))}
          for b in range(B)}

    def tile_w(w):  # [CH, E] slice of W -> [128, KT*CH] pretiled .T blocks
        wt = np.ascontiguousarray(w.T.astype(bf))          # [E, CH]
        return np.ascontiguousarray(
            wt.reshape(KT, 128, CH).transpose(1, 0, 2).reshape(128, KT * CH))

    bd16_np = np.zeros((16, 128 * KT), dtype=bf)
    for k in range(KT):
        src, h2 = divmod(k, 2)
        for c in range(128):
            bd16_np[src * 4 + h2 * 2 + c // 64, k * 128 + c] = 1

    in_maps = []
    for c in range(N_CORES):
        b, g = divmod(c, TP)
        sl = slice(g * CH, (g + 1) * CH)
        in_maps.append({
            "xqT": xT[b]["q"], "xkT": xT[b]["k"], "xvT": xT[b]["v"],
            "wqT": tile_w(Wq[sl, :]), "wkT": tile_w(Wk[sl, :]),
            "wvT": tile_w(Wv[sl, :]),
            "bqc": bq[sl], "bkc": bk[sl],
            "bvb": np.ascontiguousarray(
                np.broadcast_to(bv[sl].reshape(1, CH), (128, CH))),
            "woTb": tile_w(Wo[sl, :]),
            "bor": np.ascontiguousarray(
                np.broadcast_to(bo[sl].reshape(1, CH), (128, CH))),
            "onescol": np.ones((128, 4), dtype=bf),
            "bd16": bd16_np,
        })

    kwargs = {}
    if _trace:
        kwargs.update(trace=True,
                      trace_cores=_trace_cores or list(range(N_CORES)))
    res = run_bass_kernel_spmd(nc, in_maps, core_ids=list(range(N_CORES)),
                               **kwargs)

    full = np.empty((B, L, E), dtype=np.float32)
    for c in range(N_CORES):
        b, g = divmod(c, TP)
        full[b, :, g * CH:(g + 1) * CH] = res.results[c]["out"]

    if _trace:
        kernel.last_exec_ns = res.exec_time_ns
        kernel.last_results = res
    return full
